# revision 22
# baseline (speedup 1.0000x reference)
# Causal self-attention (B=4, T=2048, C=1024, H=16, D=64) on 8 TRN2 NeuronCores.
#
# Sharding: core c = (batch b = c//2, head-half g = c%2) -> 8 heads of one batch.
# Each core computes the qkv projection for its head group, causal attention,
# and a rank-512 partial of the output projection. Host sums the two partials
# per batch and adds the constant vector W_proj @ b_v + b_proj (the k-bias is
# dropped: softmax is invariant to it; the v-bias commutes through the convex
# combination).
#
# On-core scheme (PE operands in bf16, accumulation in fp32 PSUM):
#   qT/kT tiles [128, T] hold two heads (partitions 0-63 / 64-127). Per
#   s-tile both heads' scores land in one [128, 1024] 2-bank PSUM tile, so
#   one exp instruction covers both. exp() needs no max-subtraction
#   (|S| <~ 2). The causal triangle is applied multiplicatively on the bf16
#   probability tile (exp then zero) with a single r-independent [128,128]
#   0/1 mask - DVE runs it in 4x mode. Row sums come free from an appended
#   ones-column on V (M=65 AV matmuls); normalization = DVE reciprocal +
#   gpsimd partition_broadcast + DVE mult.
#   Schedule: scores are emitted one s-step ahead of AV so exp latency is
#   hidden; QKV chains for block n+1 and output-projection chains for block
#   n-1 are interleaved into attention block n as PE filler.
import numpy as np

B, T, C, H, D = 4, 2048, 1024, 16, 64

_NC = None


def _build(reps=1):
    import concourse.bacc as bacc
    import concourse.tile as tile
    from concourse import mybir

    BF16 = mybir.dt.bfloat16
    F32 = mybir.dt.float32
    AF = mybir.ActivationFunctionType
    ALU = mybir.AluOpType

    nc = bacc.Bacc("TRN2", target_bir_lowering=False, debug=False, num_devices=8)
    xT = nc.dram_tensor("xT", [C, T], BF16, kind="ExternalInput")
    wqT = nc.dram_tensor("wqT", [C, 512], BF16, kind="ExternalInput")
    wkT = nc.dram_tensor("wkT", [C, 512], BF16, kind="ExternalInput")
    wvT = nc.dram_tensor("wvT", [C, 512], BF16, kind="ExternalInput")
    wpT = nc.dram_tensor("wpT", [512, C], BF16, kind="ExternalInput")
    bq2 = nc.dram_tensor("bq2", [4, 128], F32, kind="ExternalInput")
    mask = nc.dram_tensor("mask", [128, 2, 128], BF16, kind="ExternalInput")
    out = nc.dram_tensor("out", [T, C], F32, kind="ExternalOutput")

    NJ = T // 512  # t blocks
    NS = T // 128  # s tiles

    with tile.TileContext(nc) as tc:
        with (
            tc.tile_pool(name="const", bufs=1) as const,
            tc.tile_pool(name="xq_p", bufs=2) as xq_p,
            tc.tile_pool(name="qt_p", bufs=8) as qt_p,
            tc.tile_pool(name="pt_p", bufs=6) as pt_p,
            tc.tile_pool(name="ot_p", bufs=12) as ot_p,
            tc.tile_pool(name="sm_p", bufs=4) as sm_p,
            tc.tile_pool(name="ob_p", bufs=4) as ob_p,
            tc.tile_pool(name="ps_a", bufs=2, space="PSUM") as ps_a,
            tc.tile_pool(name="ps_st", bufs=2, space="PSUM") as ps_st,
            tc.tile_pool(name="ps_o", bufs=1, space="PSUM") as ps_o,
        ):
            # resident weights / constants
            wq_sb = const.tile([128, 8, 512], BF16)
            wk_sb = const.tile([128, 8, 512], BF16)
            wv_sb = const.tile([128, 8, 512], BF16)
            wp_sb = const.tile([128, 4, C], BF16)
            mk_sb = const.tile([128, 2, 128], BF16)
            bq_sb = const.tile([128, 4], F32)
            ones_sb = const.tile([128, 8], BF16)
            nc.vector.memset(ones_sb[:], 1.0)
            # weights go on the scalar-engine DGE queue so they don't delay
            # the x loads on the sync queue; wp is only needed from proj(0)
            wqr = wqT.rearrange("(kt p) m -> p kt m", p=128)
            wkr = wkT.rearrange("(kt p) m -> p kt m", p=128)
            nc.scalar.dma_start(wq_sb[:, 0:4, :], wqr[:, 0:4, :])
            nc.scalar.dma_start(wk_sb[:, 0:4, :], wkr[:, 0:4, :])
            nc.scalar.dma_start(wq_sb[:, 4:8, :], wqr[:, 4:8, :])
            nc.scalar.dma_start(wk_sb[:, 4:8, :], wkr[:, 4:8, :])
            nc.scalar.dma_start(bq_sb[:], bq2.rearrange("m p -> p m"))
            nc.scalar.dma_start(mk_sb[:], mask.rearrange("p h c -> p h c"))
            nc.scalar.dma_start(wp_sb[:], wpT.rearrange("(pr p) co -> p pr co", p=128))

            kt = [
                const.tile([128, T], BF16, name=f"kt{i}", tag=f"kt{i}")
                for i in range(4)
            ]
            vt = [
                const.tile([128, 8, 65], BF16, name=f"vt{i}", tag=f"vt{i}")
                for i in range(NS)
            ]

            xTr = xT.rearrange("(kt p) t -> p kt t", p=128)

            for rep in range(reps):
                qt = {}
                xq = {}
                ot2s = {}

                def load_x(n):
                    xs = []
                    for half in range(2):
                        xh = xq_p.tile(
                            [128, 4, 512], BF16,
                            name=f"xq_{rep}_{n}_{half}", tag=f"xq{half}",
                        )
                        nc.sync.dma_start(
                            xh[:],
                            xTr[:, 4 * half : 4 * half + 4, 512 * n : 512 * (n + 1)],
                        )
                        xs.append(xh)
                    xq[n] = xs

                def q_chain(n, mt):
                    def emit():
                        psq = ps_a.tile([128, 512], F32, tag="ps_a")
                        for k in range(8):
                            nc.tensor.matmul(
                                psq[:],
                                wq_sb[:, k, 128 * mt : 128 * (mt + 1)],
                                xq[n][k // 4][:, k % 4, :],
                                start=(k == 0),
                                stop=(k == 7),
                            )
                        q_tile = qt_p.tile(
                            [128, 512], BF16, name=f"qt_{rep}_{mt}_{n}", tag="qt"
                        )
                        qt[(mt, n)] = q_tile
                        nc.vector.tensor_scalar_add(
                            q_tile[:], psq[:], bq_sb[:, mt : mt + 1]
                        )
                    return emit

                def k_chain(n, mt):
                    def emit():
                        psk = ps_a.tile([128, 512], F32, tag="ps_a")
                        for k in range(8):
                            nc.tensor.matmul(
                                psk[:],
                                wk_sb[:, k, 128 * mt : 128 * (mt + 1)],
                                xq[n][k // 4][:, k % 4, :],
                                start=(k == 0),
                                stop=(k == 7),
                            )
                        nc.vector.tensor_copy(
                            kt[mt][:, 512 * n : 512 * (n + 1)], psk[:]
                        )
                    return emit

                def v_chain(n, tt):
                    def emit():
                        psv = ps_a.tile([128, 512], F32, tag="ps_a")
                        for k in range(8):
                            nc.tensor.matmul(
                                psv[:],
                                xq[n][k // 4][:, k % 4, 128 * tt : 128 * (tt + 1)],
                                wv_sb[:, k, :],
                                start=(k == 0),
                                stop=(k == 7),
                            )
                        si = 4 * n + tt
                        nc.vector.tensor_copy(
                            vt[si][:, :, 0:64],
                            psv.rearrange("p (h d) -> p h d", d=64),
                        )
                        nc.vector.tensor_copy(vt[si][:, :, 64], ones_sb[:])
                    return emit

                def proj_chain(j, tt, half):
                    def emit():
                        pp = ps_a.tile([128, 512], F32, tag="ps_a")
                        for pair in range(4):
                            nc.tensor.matmul(
                                pp[:],
                                ot2s[j][pair][:, 128 * tt : 128 * (tt + 1)],
                                wp_sb[:, pair, 512 * half : 512 * (half + 1)],
                                start=(pair == 0),
                                stop=(pair == 3),
                            )
                        ob = ob_p.tile([128, 512], F32, tag="ob")
                        nc.vector.tensor_copy(ob[:], pp[:])
                        eng = nc.sync if half == 0 else nc.scalar
                        eng.dma_start(
                            out[
                                512 * j + 128 * tt : 512 * j + 128 * (tt + 1),
                                512 * half : 512 * (half + 1),
                            ],
                            ob[:],
                        )
                    return emit

                def qkv_chains(n):
                    ch = []
                    for mt in range(4):
                        ch.append(q_chain(n, mt))
                        ch.append(k_chain(n, mt))
                        ch.append(v_chain(n, mt))
                    return ch

                def attn_block(j, filler):
                    """Attention for t-block j; filler chains interleaved.

                    The scores->exp stream runs AHEAD si-steps in front of
                    the AV stream (pt tiles buffer the gap), so the ACT
                    engine is never throttled by AV pacing or by the
                    normalization chain at pair boundaries.
                    """
                    nfill = len(filler)
                    ns_live = 4 * (j + 1)
                    steps_total = 4 * ns_live
                    state = {"step": 0, "emitted": 0}
                    AHEAD = 3

                    def maybe_fill(extra=0):
                        state["step"] += 1
                        want = min(
                            nfill,
                            (nfill * state["step"]) // steps_total + extra,
                        )
                        while state["emitted"] < want:
                            filler[state["emitted"]]()
                            state["emitted"] += 1

                    pts = {}
                    oaugs = {}

                    def scores(pair, si):
                        r = si - 4 * j
                        off = 128 * r if r > 0 else 0
                        st2 = ps_st.tile([128, 1024], F32, tag="st")
                        for h in range(2):
                            nc.tensor.matmul(
                                st2[:, 512 * h + off : 512 * h + 512],
                                kt[pair][
                                    64 * h : 64 * h + 64,
                                    128 * si : 128 * si + 128,
                                ],
                                qt[(pair, j)][64 * h : 64 * h + 64, off:],
                                start=True,
                                stop=True,
                            )
                        pt2 = pt_p.tile([128, 1024], BF16, tag="pt")
                        pts[(pair, si)] = pt2
                        st2v = st2.rearrange("p (h c) -> p h c", h=2)
                        pt2v = pt2.rearrange("p (h c) -> p h c", h=2)
                        nc.scalar.activation(
                            pt2v[:, :, off:], st2v[:, :, off:], AF.Exp
                        )
                        if r >= 0:
                            nc.vector.tensor_tensor(
                                pt2v[:, :, off : off + 128],
                                pt2v[:, :, off : off + 128],
                                mk_sb[:],
                                ALU.mult,
                            )

                    def av(pair, si):
                        # Diagonal tiles split: cols past the masked
                        # triangle depend only on exp; the 128-wide
                        # triangle block additionally waits for the DVE
                        # mask-mult, so it is issued second.
                        r = si - 4 * j
                        off = 128 * r if r > 0 else 0
                        if si == 0:
                            oaugs[pair] = [
                                ps_o.tile(
                                    [65, 512], F32,
                                    name=f"oaug_{rep}_{j}_{pair}_{h}",
                                    tag=f"ps_o{h}",
                                )
                                for h in range(2)
                            ]
                        oaug = oaugs[pair]
                        pt2 = pts.pop((pair, si))
                        last = si == ns_live - 1
                        if r >= 0 and si > 0:
                            for h in range(2):
                                if off + 128 < 512:
                                    nc.tensor.matmul(
                                        oaug[h][:, off + 128 :],
                                        vt[si][:, 2 * pair + h, :],
                                        pt2[:, 512 * h + off + 128 : 512 * h + 512],
                                        start=(si == 0),
                                        stop=False,
                                    )
                            for h in range(2):
                                nc.tensor.matmul(
                                    oaug[h][:, off : off + 128],
                                    vt[si][:, 2 * pair + h, :],
                                    pt2[:, 512 * h + off : 512 * h + off + 128],
                                    start=(si == 0),
                                    stop=last,
                                )
                        else:
                            for h in range(2):
                                nc.tensor.matmul(
                                    oaug[h][:, off:],
                                    vt[si][:, 2 * pair + h, :],
                                    pt2[:, 512 * h + off : 512 * h + 512],
                                    start=(si == 0),
                                    stop=last,
                                )

                    def norm(pair):
                        # On the final block the norm->proj chain is exposed,
                        # so it is chunked by tt-column to shorten the
                        # latency to the first proj matmul.
                        oaug = oaugs.pop(pair)
                        o_tile = ot_p.tile(
                            [128, 512], BF16, name=f"ot_{rep}_{pair}_{j}", tag="ot"
                        )
                        ot2s[j].append(o_tile)
                        chunks = [(0, 512)]
                        for h in range(2):
                            for lo, hi in chunks:
                                rec = sm_p.tile([1, 512], F32, tag="rec")
                                nc.vector.reciprocal(
                                    rec[:, lo:hi], oaug[h][64:65, lo:hi]
                                )
                                bc = sm_p.tile([64, 512], F32, tag="bc")
                                nc.gpsimd.partition_broadcast(
                                    bc[:, lo:hi], rec[:, lo:hi]
                                )
                                nc.vector.tensor_tensor(
                                    o_tile[64 * h : 64 * h + 64, lo:hi],
                                    oaug[h][0:64, lo:hi],
                                    bc[:, lo:hi],
                                    ALU.mult,
                                )

                    ot2s[j] = []
                    work = [(p, si) for p in range(4) for si in range(ns_live)]
                    sp = 0
                    for idx, (p, si) in enumerate(work):
                        while sp < len(work) and sp <= idx + AHEAD:
                            scores(*work[sp])
                            sp += 1
                        maybe_fill()
                        av(p, si)
                        if si == ns_live - 1:
                            norm(p)

                    for f in filler[state["emitted"] :]:
                        f()

                # prologue: x block 0 + its qkv; wv rides the sync queue
                # right behind x(0) so v-chains unblock early
                load_x(0)
                if rep == 0:
                    nc.sync.dma_start(
                        wv_sb[:], wvT.rearrange("(kt p) m -> p kt m", p=128)
                    )
                for ch in qkv_chains(0):
                    ch()

                # proj chains for block d are used as filler in block
                # fill_at[d]: late attn blocks have the largest exp deficit,
                # so proj work is pushed there.
                fill_at = {0: 2, 1: 3, 2: 3}
                for j in range(NJ):
                    filler = []
                    if j + 1 < NJ:
                        load_x(j + 1)
                        nxt = qkv_chains(j + 1)
                    else:
                        nxt = []
                    prj = [
                        proj_chain(d, tt, half)
                        for d, fj in fill_at.items()
                        if fj == j
                        for tt in range(4)
                        for half in range(2)
                    ]
                    # interleave qkv and proj chains evenly
                    na, nb = len(nxt), len(prj)
                    ia = ib = 0
                    for s in range(na + nb):
                        if ib >= nb or (ia < na and ia * nb <= ib * na):
                            filler.append(nxt[ia]); ia += 1
                        else:
                            filler.append(prj[ib]); ib += 1
                    attn_block(j, filler)

                for tt in range(4):
                    for half in range(2):
                        proj_chain(NJ - 1, tt, half)()
    nc.compile()
    return nc


def _get_nc():
    global _NC
    if _NC is None:
        _NC = _build()
    return _NC


def _host_mask():
    import ml_dtypes

    p = np.arange(128)[:, None]
    c = np.arange(128)[None, :]
    m = (c >= p).astype(np.float32)
    return np.broadcast_to(m[:, None, :], (128, 2, 128)).astype(ml_dtypes.bfloat16)


def _in_maps(x, W_attn, b_attn, W_proj):
    import ml_dtypes

    bf16 = ml_dtypes.bfloat16
    Wq, Wk, Wv = W_attn[0:C], W_attn[C : 2 * C], W_attn[2 * C : 3 * C]
    mask = _host_mask()
    g_in = []
    for g in range(2):
        sl = slice(512 * g, 512 * (g + 1))
        g_in.append(
            dict(
                wqT=(np.ascontiguousarray(Wq[sl].T) * 0.125).astype(bf16),
                wkT=np.ascontiguousarray(Wk[sl].T).astype(bf16),
                wvT=np.ascontiguousarray(Wv[sl].T).astype(bf16),
                wpT=np.ascontiguousarray(W_proj[:, sl].T).astype(bf16),
                bq2=(b_attn[sl] * 0.125).reshape(4, 128).astype(np.float32),
                mask=mask,
            )
        )
    xTs = [np.ascontiguousarray(x[b].T).astype(bf16) for b in range(B)]
    return [dict(xT=xTs[c // 2], **g_in[c % 2]) for c in range(8)]


def _run(x, W_attn, b_attn, W_proj, b_proj, **rk):
    from concourse.bass_utils import run_bass_kernel_spmd

    x = np.asarray(x, dtype=np.float32)
    W_attn = np.asarray(W_attn, dtype=np.float32)
    b_attn = np.asarray(b_attn, dtype=np.float32)
    W_proj = np.asarray(W_proj, dtype=np.float32)
    b_proj = np.asarray(b_proj, dtype=np.float32)

    nc = _get_nc()
    in_maps = _in_maps(x, W_attn, b_attn, W_proj)
    res = run_bass_kernel_spmd(nc, in_maps, core_ids=list(range(8)), **rk)

    cvec = (W_proj @ b_attn[2 * C : 3 * C] + b_proj).astype(np.float32)
    y = np.empty((B, T, C), np.float32)
    for b in range(B):
        y[b] = res.results[2 * b]["out"] + res.results[2 * b + 1]["out"] + cvec
    return y, res


def kernel(x, W_attn, b_attn, W_proj, b_proj):
    return _run(x, W_attn, b_attn, W_proj, b_proj)[0]


# revision 25
# speedup vs baseline: 1.0213x; 1.0213x over previous
# Causal self-attention (B=4, T=2048, C=1024, H=16, D=64) on 8 TRN2 NeuronCores.
#
# Sharding: core c = (batch b = c//2, head-half g = c%2) -> 8 heads of one batch.
# Each core computes the qkv projection for its head group, causal attention,
# and a rank-512 partial of the output projection. Host sums the two partials
# per batch and adds the constant vector W_proj @ b_v + b_proj (the k-bias is
# dropped: softmax is invariant to it; the v-bias commutes through the convex
# combination).
#
# On-core scheme (PE operands in bf16, accumulation in fp32 PSUM):
#   qT/kT tiles [128, T] hold two heads (partitions 0-63 / 64-127). Per
#   s-tile both heads' scores land in one [128, 1024] 2-bank PSUM tile, so
#   one exp instruction covers both. exp() needs no max-subtraction
#   (|S| <~ 2). The causal triangle is applied multiplicatively on the bf16
#   probability tile (exp then zero) with a single r-independent [128,128]
#   0/1 mask - DVE runs it in 4x mode. Row sums come free from an appended
#   ones-column on V (M=65 AV matmuls); normalization = DVE reciprocal +
#   gpsimd partition_broadcast + DVE mult.
#   Schedule: scores are emitted one s-step ahead of AV so exp latency is
#   hidden; QKV chains for block n+1 and output-projection chains for block
#   n-1 are interleaved into attention block n as PE filler.
import numpy as np

B, T, C, H, D = 4, 2048, 1024, 16, 64

_NC = None


def _build(reps=1):
    import concourse.bacc as bacc
    import concourse.tile as tile
    from concourse import mybir

    BF16 = mybir.dt.bfloat16
    F32 = mybir.dt.float32
    AF = mybir.ActivationFunctionType
    ALU = mybir.AluOpType

    nc = bacc.Bacc("TRN2", target_bir_lowering=False, debug=False, num_devices=8)
    xT = nc.dram_tensor("xT", [C, T], BF16, kind="ExternalInput")
    wqT = nc.dram_tensor("wqT", [C, 512], BF16, kind="ExternalInput")
    wkT = nc.dram_tensor("wkT", [C, 512], BF16, kind="ExternalInput")
    wvT = nc.dram_tensor("wvT", [C, 512], BF16, kind="ExternalInput")
    wpT = nc.dram_tensor("wpT", [512, C], BF16, kind="ExternalInput")
    bq2 = nc.dram_tensor("bq2", [4, 128], F32, kind="ExternalInput")
    mask = nc.dram_tensor("mask", [128, 2, 128], BF16, kind="ExternalInput")
    out = nc.dram_tensor("out", [T, C], F32, kind="ExternalOutput")

    NJ = T // 512  # t blocks
    NS = T // 128  # s tiles

    with tile.TileContext(nc) as tc:
        with (
            tc.tile_pool(name="const", bufs=1) as const,
            tc.tile_pool(name="xq_p", bufs=2) as xq_p,
            tc.tile_pool(name="qt_p", bufs=8) as qt_p,
            tc.tile_pool(name="pt_p", bufs=6) as pt_p,
            tc.tile_pool(name="ot_p", bufs=12) as ot_p,
            tc.tile_pool(name="sm_p", bufs=4) as sm_p,
            tc.tile_pool(name="ob_p", bufs=4) as ob_p,
            tc.tile_pool(name="ps_a", bufs=2, space="PSUM") as ps_a,
            tc.tile_pool(name="ps_st", bufs=2, space="PSUM") as ps_st,
            tc.tile_pool(name="ps_o", bufs=1, space="PSUM") as ps_o,
        ):
            # resident weights / constants
            wq_sb = const.tile([128, 8, 512], BF16)
            wk_sb = const.tile([128, 8, 512], BF16)
            wv_sb = const.tile([128, 8, 512], BF16)
            wp_sb = const.tile([128, 4, C], BF16)
            mk_sb = const.tile([128, 2, 128], BF16)
            bq_sb = const.tile([128, 4], F32)
            ones_sb = const.tile([128, 8], BF16)
            nc.vector.memset(ones_sb[:], 1.0)
            # weights ride the scalar-engine DGE queue in consumption-order
            # 2-slice pieces so the first q/k chains start ~3us in and never
            # micro-stall (each stall resets the PE p-state ramp); x rides
            # the sync queue in matching pieces. wp is only needed at proj(0)
            wqr = wqT.rearrange("(kt p) m -> p kt m", p=128)
            wkr = wkT.rearrange("(kt p) m -> p kt m", p=128)
            for a in range(0, 8, 2):
                nc.scalar.dma_start(wq_sb[:, a : a + 2, :], wqr[:, a : a + 2, :])
            nc.scalar.dma_start(bq_sb[:], bq2.rearrange("m p -> p m"))
            for a in range(0, 8, 2):
                nc.scalar.dma_start(wk_sb[:, a : a + 2, :], wkr[:, a : a + 2, :])
            nc.scalar.dma_start(mk_sb[:], mask.rearrange("p h c -> p h c"))
            nc.scalar.dma_start(wp_sb[:], wpT.rearrange("(pr p) co -> p pr co", p=128))

            kt = [
                const.tile([128, T], BF16, name=f"kt{i}", tag=f"kt{i}")
                for i in range(4)
            ]
            vt = [
                const.tile([128, 8, 65], BF16, name=f"vt{i}", tag=f"vt{i}")
                for i in range(NS)
            ]

            xTr = xT.rearrange("(kt p) t -> p kt t", p=128)

            for rep in range(reps):
                qt = {}
                xq = {}
                ot2s = {}

                def load_x(n, pieces=1):
                    xs = []
                    for half in range(2):
                        xh = xq_p.tile(
                            [128, 4, 512], BF16,
                            name=f"xq_{rep}_{n}_{half}", tag=f"xq{half}",
                        )
                        step = 4 // pieces
                        for a in range(0, 4, step):
                            nc.sync.dma_start(
                                xh[:, a : a + step, :],
                                xTr[
                                    :,
                                    4 * half + a : 4 * half + a + step,
                                    512 * n : 512 * (n + 1),
                                ],
                            )
                        xs.append(xh)
                    xq[n] = xs

                def q_chain(n, mt):
                    def emit():
                        psq = ps_a.tile([128, 512], F32, tag="ps_a")
                        for k in range(8):
                            nc.tensor.matmul(
                                psq[:],
                                wq_sb[:, k, 128 * mt : 128 * (mt + 1)],
                                xq[n][k // 4][:, k % 4, :],
                                start=(k == 0),
                                stop=(k == 7),
                            )
                        q_tile = qt_p.tile(
                            [128, 512], BF16, name=f"qt_{rep}_{mt}_{n}", tag="qt"
                        )
                        qt[(mt, n)] = q_tile
                        nc.vector.tensor_scalar_add(
                            q_tile[:], psq[:], bq_sb[:, mt : mt + 1]
                        )
                    return emit

                def k_chain(n, mt):
                    def emit():
                        psk = ps_a.tile([128, 512], F32, tag="ps_a")
                        for k in range(8):
                            nc.tensor.matmul(
                                psk[:],
                                wk_sb[:, k, 128 * mt : 128 * (mt + 1)],
                                xq[n][k // 4][:, k % 4, :],
                                start=(k == 0),
                                stop=(k == 7),
                            )
                        nc.vector.tensor_copy(
                            kt[mt][:, 512 * n : 512 * (n + 1)], psk[:]
                        )
                    return emit

                def v_chain(n, tt):
                    def emit():
                        psv = ps_a.tile([128, 512], F32, tag="ps_a")
                        for k in range(8):
                            nc.tensor.matmul(
                                psv[:],
                                xq[n][k // 4][:, k % 4, 128 * tt : 128 * (tt + 1)],
                                wv_sb[:, k, :],
                                start=(k == 0),
                                stop=(k == 7),
                            )
                        si = 4 * n + tt
                        nc.vector.tensor_copy(
                            vt[si][:, :, 0:64],
                            psv.rearrange("p (h d) -> p h d", d=64),
                        )
                        nc.vector.tensor_copy(vt[si][:, :, 64], ones_sb[:])
                    return emit

                def proj_chain(j, tt, half):
                    def emit():
                        pp = ps_a.tile([128, 512], F32, tag="ps_a")
                        for pair in range(4):
                            nc.tensor.matmul(
                                pp[:],
                                ot2s[j][pair][:, 128 * tt : 128 * (tt + 1)],
                                wp_sb[:, pair, 512 * half : 512 * (half + 1)],
                                start=(pair == 0),
                                stop=(pair == 3),
                            )
                        ob = ob_p.tile([128, 512], F32, tag="ob")
                        nc.vector.tensor_copy(ob[:], pp[:])
                        eng = nc.sync if half == 0 else nc.scalar
                        eng.dma_start(
                            out[
                                512 * j + 128 * tt : 512 * j + 128 * (tt + 1),
                                512 * half : 512 * (half + 1),
                            ],
                            ob[:],
                        )
                    return emit

                def qkv_chains(n):
                    ch = []
                    for mt in range(4):
                        ch.append(q_chain(n, mt))
                        ch.append(k_chain(n, mt))
                        ch.append(v_chain(n, mt))
                    return ch

                def attn_block(j, filler):
                    """Attention for t-block j; filler chains interleaved.

                    The scores->exp stream runs AHEAD si-steps in front of
                    the AV stream (pt tiles buffer the gap), so the ACT
                    engine is never throttled by AV pacing or by the
                    normalization chain at pair boundaries.
                    """
                    nfill = len(filler)
                    ns_live = 4 * (j + 1)
                    steps_total = 4 * ns_live
                    state = {"step": 0, "emitted": 0}
                    AHEAD = 3

                    # filler must be fully emitted a few steps before the
                    # block ends: a leftover chain's PSUM-bank WAR would
                    # stall the first chains of the next phase
                    fill_span = max(1, steps_total - 6)

                    def maybe_fill(extra=0):
                        state["step"] += 1
                        want = min(
                            nfill,
                            (nfill * state["step"]) // fill_span + extra,
                        )
                        while state["emitted"] < want:
                            filler[state["emitted"]]()
                            state["emitted"] += 1

                    pts = {}
                    oaugs = {}

                    def scores(pair, si):
                        r = si - 4 * j
                        off = 128 * r if r > 0 else 0
                        st2 = ps_st.tile([128, 1024], F32, tag="st")
                        for h in range(2):
                            nc.tensor.matmul(
                                st2[:, 512 * h + off : 512 * h + 512],
                                kt[pair][
                                    64 * h : 64 * h + 64,
                                    128 * si : 128 * si + 128,
                                ],
                                qt[(pair, j)][64 * h : 64 * h + 64, off:],
                                start=True,
                                stop=True,
                            )
                        pt2 = pt_p.tile([128, 1024], BF16, tag="pt")
                        pts[(pair, si)] = pt2
                        st2v = st2.rearrange("p (h c) -> p h c", h=2)
                        pt2v = pt2.rearrange("p (h c) -> p h c", h=2)
                        nc.scalar.activation(
                            pt2v[:, :, off:], st2v[:, :, off:], AF.Exp
                        )
                        if r >= 0:
                            nc.vector.tensor_tensor(
                                pt2v[:, :, off : off + 128],
                                pt2v[:, :, off : off + 128],
                                mk_sb[:],
                                ALU.mult,
                            )

                    def av(pair, si):
                        # Diagonal tiles split: cols past the masked
                        # triangle depend only on exp; the 128-wide
                        # triangle block additionally waits for the DVE
                        # mask-mult, so it is issued second.
                        r = si - 4 * j
                        off = 128 * r if r > 0 else 0
                        if si == 0:
                            oaugs[pair] = [
                                ps_o.tile(
                                    [65, 512], F32,
                                    name=f"oaug_{rep}_{j}_{pair}_{h}",
                                    tag=f"ps_o{h}",
                                )
                                for h in range(2)
                            ]
                        oaug = oaugs[pair]
                        pt2 = pts.pop((pair, si))
                        last = si == ns_live - 1
                        if r >= 0 and si > 0:
                            for h in range(2):
                                if off + 128 < 512:
                                    nc.tensor.matmul(
                                        oaug[h][:, off + 128 :],
                                        vt[si][:, 2 * pair + h, :],
                                        pt2[:, 512 * h + off + 128 : 512 * h + 512],
                                        start=(si == 0),
                                        stop=False,
                                    )
                            for h in range(2):
                                nc.tensor.matmul(
                                    oaug[h][:, off : off + 128],
                                    vt[si][:, 2 * pair + h, :],
                                    pt2[:, 512 * h + off : 512 * h + off + 128],
                                    start=(si == 0),
                                    stop=last,
                                )
                        else:
                            for h in range(2):
                                nc.tensor.matmul(
                                    oaug[h][:, off:],
                                    vt[si][:, 2 * pair + h, :],
                                    pt2[:, 512 * h + off : 512 * h + 512],
                                    start=(si == 0),
                                    stop=last,
                                )

                    def norm(pair):
                        # On the final block the norm->proj chain is exposed,
                        # so it is chunked by tt-column to shorten the
                        # latency to the first proj matmul.
                        oaug = oaugs.pop(pair)
                        o_tile = ot_p.tile(
                            [128, 512], BF16, name=f"ot_{rep}_{pair}_{j}", tag="ot"
                        )
                        ot2s[j].append(o_tile)
                        chunks = [(0, 512)]
                        for h in range(2):
                            for lo, hi in chunks:
                                rec = sm_p.tile([1, 512], F32, tag="rec")
                                nc.vector.reciprocal(
                                    rec[:, lo:hi], oaug[h][64:65, lo:hi]
                                )
                                bc = sm_p.tile([64, 512], F32, tag="bc")
                                nc.gpsimd.partition_broadcast(
                                    bc[:, lo:hi], rec[:, lo:hi]
                                )
                                nc.vector.tensor_tensor(
                                    o_tile[64 * h : 64 * h + 64, lo:hi],
                                    oaug[h][0:64, lo:hi],
                                    bc[:, lo:hi],
                                    ALU.mult,
                                )

                    ot2s[j] = []
                    work = [(p, si) for p in range(4) for si in range(ns_live)]
                    sp = 0
                    for idx, (p, si) in enumerate(work):
                        while sp < len(work) and sp <= idx + AHEAD:
                            scores(*work[sp])
                            sp += 1
                        maybe_fill()
                        av(p, si)
                        if si == ns_live - 1:
                            norm(p)

                    for f in filler[state["emitted"] :]:
                        f()

                # prologue: x block 0 + its qkv; wv rides the sync queue
                # right behind x(0) so v-chains unblock early. Chains run
                # q,q,q,q,k,...,v order: each weight tensor arrives while
                # the previous chain type computes.
                load_x(0, pieces=2)
                if rep == 0:
                    wvr = wvT.rearrange("(kt p) m -> p kt m", p=128)
                    nc.sync.dma_start(wv_sb[:, 0:4, :], wvr[:, 0:4, :])
                    nc.sync.dma_start(wv_sb[:, 4:8, :], wvr[:, 4:8, :])
                for mt in range(4):
                    q_chain(0, mt)()
                for mt in range(4):
                    k_chain(0, mt)()
                for mt in range(4):
                    v_chain(0, mt)()

                # proj chains for block d are used as filler in block
                # fill_at[d]: late attn blocks have the largest exp deficit,
                # so proj work is pushed there.
                fill_at = {0: 2, 1: 3, 2: 3}
                for j in range(NJ):
                    filler = []
                    if j + 1 < NJ:
                        load_x(j + 1)
                        nxt = qkv_chains(j + 1)
                    else:
                        nxt = []
                    prj = [
                        proj_chain(d, tt, half)
                        for d, fj in fill_at.items()
                        if fj == j
                        for tt in range(4)
                        for half in range(2)
                    ]
                    # interleave qkv and proj chains evenly
                    na, nb = len(nxt), len(prj)
                    ia = ib = 0
                    for s in range(na + nb):
                        if ib >= nb or (ia < na and ia * nb <= ib * na):
                            filler.append(nxt[ia]); ia += 1
                        else:
                            filler.append(prj[ib]); ib += 1
                    attn_block(j, filler)

                for tt in range(4):
                    for half in range(2):
                        proj_chain(NJ - 1, tt, half)()
    nc.compile()
    return nc


def _get_nc():
    global _NC
    if _NC is None:
        _NC = _build()
    return _NC


def _host_mask():
    import ml_dtypes

    p = np.arange(128)[:, None]
    c = np.arange(128)[None, :]
    m = (c >= p).astype(np.float32)
    return np.broadcast_to(m[:, None, :], (128, 2, 128)).astype(ml_dtypes.bfloat16)


def _in_maps(x, W_attn, b_attn, W_proj):
    import ml_dtypes

    bf16 = ml_dtypes.bfloat16
    Wq, Wk, Wv = W_attn[0:C], W_attn[C : 2 * C], W_attn[2 * C : 3 * C]
    mask = _host_mask()
    g_in = []
    for g in range(2):
        sl = slice(512 * g, 512 * (g + 1))
        g_in.append(
            dict(
                wqT=(np.ascontiguousarray(Wq[sl].T) * 0.125).astype(bf16),
                wkT=np.ascontiguousarray(Wk[sl].T).astype(bf16),
                wvT=np.ascontiguousarray(Wv[sl].T).astype(bf16),
                wpT=np.ascontiguousarray(W_proj[:, sl].T).astype(bf16),
                bq2=(b_attn[sl] * 0.125).reshape(4, 128).astype(np.float32),
                mask=mask,
            )
        )
    xTs = [np.ascontiguousarray(x[b].T).astype(bf16) for b in range(B)]
    return [dict(xT=xTs[c // 2], **g_in[c % 2]) for c in range(8)]


def _run(x, W_attn, b_attn, W_proj, b_proj, **rk):
    from concourse.bass_utils import run_bass_kernel_spmd

    x = np.asarray(x, dtype=np.float32)
    W_attn = np.asarray(W_attn, dtype=np.float32)
    b_attn = np.asarray(b_attn, dtype=np.float32)
    W_proj = np.asarray(W_proj, dtype=np.float32)
    b_proj = np.asarray(b_proj, dtype=np.float32)

    nc = _get_nc()
    in_maps = _in_maps(x, W_attn, b_attn, W_proj)
    res = run_bass_kernel_spmd(nc, in_maps, core_ids=list(range(8)), **rk)

    cvec = (W_proj @ b_attn[2 * C : 3 * C] + b_proj).astype(np.float32)
    y = np.empty((B, T, C), np.float32)
    for b in range(B):
        y[b] = res.results[2 * b]["out"] + res.results[2 * b + 1]["out"] + cvec
    return y, res


def kernel(x, W_attn, b_attn, W_proj, b_proj):
    return _run(x, W_attn, b_attn, W_proj, b_proj)[0]
